# revision 5
# baseline (speedup 1.0000x reference)
"""Baichuan-13B attention block (QKV packed proj + ALiBi causal attention via
identity paged-KV roundtrip + o_proj), tensor-parallel over 8 TRN2 NeuronCores.

Sharding: heads are split 5-per-core (w_pack column shards per interleaved
q/k/v head groups, o_proj row shards); attention outputs are AllGathered
per-batch in a feature-major (D-major / transposed) layout, and each core
computes a disjoint 640-column slice of the final output, concatenated on the
host.

The paged-KV cache fill + gather in the reference is an identity mapping:
the caches start zeroed, the block table (fill=arange) is injective, and the
gather reads back exactly the freshly written K/V. So attention consumes the
projected K/V directly.

All matmuls run in bf16 (fp32 PSUM accumulation). Softmax uses the exact
max-free rewrite exp(s + slope*(k-q)). For the three full-window head slots
the per-q shift is replaced by a per-(k-chunk, q-tile) CONSTANT shift
slope*(128i + kk - 512j - 256) folded into the exp's per-partition bias: the
column-constant part cancels in the softmax normalization, so no extra matmul
is needed and the fp32/bf16 range stays bounded (|arg| <= ~63). For the two
windowed slots (large slopes) the per-q shift -slope*q is injected into the
scores PSUM by a K=1 bf16 broadcast matmul (ones^T x rowvec) as before. The
causal mask is additive (-1e9) on diagonal blocks, applied pre-exp on DVE.

ALiBi sparsity: for slope s, keys further than ~124/s behind the query
underflow to exactly 0 in fp32 exp (both here and in the reference), so those
score blocks are skipped. Since the SPMD graph is shared by all cores, heads
are ranked by their window and dealt round-robin so every core holds one head
from each of 5 window classes; the per-slot windows are hardcoded and the
host permutes w_pack head shards / o_proj columns to match.

Scheduling: the QKV projection keeps hT slabs stationary in SBUF and STREAMS
the weights (re-read per token tile) — this frees enough SBUF to prefetch the
attention phase's batch-0 inputs during the projection's second half, and the
first matmul can start as soon as one slab chunk + one weight chunk land.
Stores are issued from the scalar (projection) and gpsimd (attention/o_proj)
DGE queues so the sync-queue loads are never FIFO-blocked behind them.
"""

import math

import numpy as np
import ml_dtypes

import concourse.bass as bass
import concourse.mybir as mybir
import concourse.tile as tile
from concourse import bacc
from concourse.bass_utils import run_bass_kernel_spmd

# ---- problem constants (hardcoded per contract) ----
B, S = 2, 2048
HID, H, D = 5120, 40, 128
N_CORES = 8
HL = H // N_CORES            # 5 local heads
FL = HL * D                  # 640 local features
T = B * S                    # 4096 tokens
SCALE = 1.0 / math.sqrt(D)

BF16 = mybir.dt.bfloat16
F32 = mybir.dt.float32
NPBF16 = ml_dtypes.bfloat16

LAST_EXEC_NS = None


def _alibi_slopes(n):
    def pow2_slopes(m):
        start = 2.0 ** (-(2.0 ** -(math.log2(m) - 3)))
        return [start * (start ** i) for i in range(m)]
    if math.log2(n).is_integer():
        return pow2_slopes(int(n))
    m = 2 ** math.floor(math.log2(n))
    return pow2_slopes(m) + pow2_slopes(2 * m)[0::2][: n - m]


def _build_nc():
    nc = bacc.Bacc(num_devices=N_CORES)

    hT = nc.declare_dram_parameter("hT", [HID, T], BF16, isOutput=False)
    wqkT = nc.declare_dram_parameter("wqkT", [HID, 2 * FL], BF16, isOutput=False)
    wvT = nc.declare_dram_parameter("wvT", [HID, FL], BF16, isOutput=False)
    owT = nc.declare_dram_parameter("owT", [HID, FL], BF16, isOutput=False)
    rowvec = nc.declare_dram_parameter("rowvec", [HL, S], BF16, isOutput=False)
    biascol = nc.declare_dram_parameter(
        "biascol", [HL, 128, (S // 128) * 4], F32, isOutput=False)
    masks = nc.declare_dram_parameter("masks", [4, 128, 512], F32, isOutput=False)
    onesM = nc.declare_dram_parameter("onesM", [128, 128], BF16, isOutput=False)
    out = nc.declare_dram_parameter("out", [T, FL], F32, isOutput=True)

    # internal DRAM scratch
    qkT = nc.dram_tensor("qkT", [2 * FL, T], BF16)          # rows: [q feats | k feats]
    # V per head, already in the B-phase SBUF layout: [hl, b, part, outer, D]
    vtok = nc.dram_tensor("vtok", [HL, B, 128, S // 128, D], BF16)
    attnT_local = [nc.dram_tensor(f"attnT_local{b}", [FL, S], BF16) for b in range(B)]
    attnT_full = [
        nc.dram_tensor(f"attnT_full{b}", [H * D, S], BF16, addr_space="Shared")
        for b in range(B)
    ]

    CT = HID // 128  # 40 contraction chunks
    NTT = T // 512   # 8 token tiles of 512
    NKC = S // 128   # 16 k-chunks per sequence

    def i_min(j, win):
        if win >= S:
            return 0
        return max(0, -(-(512 * j - win - 127) // 128))

    with tile.TileContext(nc) as tc:
        with (
            # outer scope: B-phase inputs + constants, live through A so the
            # batch-0 attention inputs can prefetch during the projection
            tc.tile_pool(name="constB", bufs=1) as cpool,
            tc.tile_pool(name="ioB", bufs=3) as iopool,
        ):
            masks_sb = cpool.tile([128, 4, 512], F32, name="masks_sb")
            nc.sync.dma_start(masks_sb[:], masks[:].rearrange("m p q -> p m q"))
            onesM_sb = cpool.tile([128, 128], BF16, name="onesM_sb")
            nc.sync.dma_start(onesM_sb[:], onesM[:])
            # windowed-slot row vectors at base partitions 0 and 32 (the
            # K=1 matmul's stationary operand must start at partition 0/32/64)
            rvs = cpool.tile([33, S], BF16, name="rvs")
            nc.sync.dma_start(rvs[0:1, :], rowvec[0:1, :])
            nc.sync.dma_start(rvs[32:33, :], rowvec[1:2, :])
            bcs = cpool.tile([128, HL, NKC * 4], F32, name="bcs")
            nc.sync.dma_start(bcs[:], biascol[:].rearrange("h p x -> p h x"))

            bt = {}  # (b, hl) -> prefetched input tiles for phase B

            def load_b(b, hl):
                kTt = iopool.tile([128, S], BF16, tag="kTt", name=f"kTt{hl}_{b}")
                nc.sync.dma_start(
                    kTt[:],
                    qkT[FL + 128 * hl: FL + 128 * (hl + 1), S * b:S * (b + 1)],
                )
                qTt = iopool.tile([128, S], BF16, tag="qTt", name=f"qTt{hl}_{b}")
                nc.sync.dma_start(
                    qTt[:], qkT[128 * hl:128 * (hl + 1), S * b:S * (b + 1)]
                )
                vt = iopool.tile([128, NKC, D], BF16, tag="vt", name=f"vt{hl}_{b}")
                nc.sync.dma_start(vt[:], vtok[hl, b])
                bt[(b, hl)] = (kTt, qTt, vt)

            # ---------- Phase A: merged Q+K+V projection (weights stream) ----------
            with (
                tc.tile_pool(name="sA", bufs=2) as spool,
                tc.tile_pool(name="wR", bufs=1) as wrpool,
                tc.tile_pool(name="wA", bufs=3) as wpool,
                tc.tile_pool(name="pA", bufs=4, space="PSUM") as ppool,
                tc.tile_pool(name="pV", bufs=2, space="PSUM") as pvpool,
                tc.tile_pool(name="eA", bufs=2) as epool,
                tc.tile_pool(name="eV", bufs=2) as evpool,
            ):
                wv = wrpool.tile([128, CT, FL], BF16, name="wv")
                for qq in range(4):
                    nc.scalar.dma_start(
                        wv[:, 10 * qq:10 * (qq + 1), :],
                        wvT[1280 * qq:1280 * (qq + 1), :].rearrange(
                            "(o p) f -> p o f", p=128),
                    )
                for tt in range(NTT):
                    slab = spool.tile([128, CT, 512], BF16, tag="slab",
                                      name=f"slab{tt}")
                    for ch in range(5):
                        nc.sync.dma_start(
                            slab[:, 8 * ch:8 * (ch + 1), :],
                            hT[1024 * ch:1024 * (ch + 1),
                               512 * tt:512 * (tt + 1)].rearrange(
                                "(o p) t -> p o t", p=128),
                        )
                    # Q/K features in groups (4, 4, 2) sharing 4 PSUM banks
                    for gi, (f0, nf) in enumerate(((0, 4), (4, 4), (8, 2))):
                        pss = [ppool.tile([128, 512], F32, tag="ps",
                                          name=f"psA{tt}_{f0}_{f}")
                               for f in range(nf)]
                        for ctb in range(10):
                            wq = wpool.tile([128, 4, 512], BF16, tag="wqk",
                                            name=f"wq{tt}_{gi}_{ctb}")
                            nc.sync.dma_start(
                                wq[:, :, 0:128 * nf],
                                wqkT[512 * ctb:512 * (ctb + 1),
                                     128 * f0:128 * f0 + 128 * nf
                                     ].rearrange("(o p) f -> p o f", p=128),
                            )
                            for cl in range(4):
                                ct = 4 * ctb + cl
                                for f in range(nf):
                                    nc.tensor.matmul(
                                        pss[f][:],
                                        wq[:, cl, 128 * f:128 * (f + 1)],
                                        slab[:, ct, :],
                                        start=(ct == 0),
                                        stop=(ct == CT - 1),
                                    )
                        for f in range(nf):
                            ft = f0 + f
                            ev = epool.tile([128, 512], BF16, tag="ev",
                                            name=f"evA{tt}_{ft}")
                            nc.scalar.copy(ev[:], pss[f][:])
                            nc.scalar.dma_start(
                                qkT[128 * ft:128 * (ft + 1),
                                    512 * tt:512 * (tt + 1)],
                                ev[:],
                            )
                    # V features, token-major (slab chunks stationary)
                    for tc4 in range(4):
                        psv = pvpool.tile([128, FL], F32, tag="psv",
                                          name=f"psv{tt}_{tc4}")
                        for ct in range(CT):
                            nc.tensor.matmul(
                                psv[:, 0:512],
                                slab[:, ct, 128 * tc4:128 * (tc4 + 1)],
                                wv[:, ct, 0:512],
                                start=(ct == 0), stop=(ct == CT - 1),
                            )
                            nc.tensor.matmul(
                                psv[:, 512:FL],
                                slab[:, ct, 128 * tc4:128 * (tc4 + 1)],
                                wv[:, ct, 512:FL],
                                start=(ct == 0), stop=(ct == CT - 1),
                            )
                        evv = evpool.tile([128, FL], BF16, tag="evv",
                                          name=f"evv{tt}_{tc4}")
                        nc.scalar.copy(evv[:], psv[:])
                        tglob = 4 * tt + tc4
                        bb, oo = tglob // (S // 128), tglob % (S // 128)
                        for hl in range(HL):
                            nc.scalar.dma_start(
                                vtok[hl, bb, :, oo, :],
                                evv[:, 128 * hl:128 * (hl + 1)],
                            )
                    # prefetch batch-0 attention inputs during the b=1 tiles
                    if 4 <= tt <= 6:
                        load_b(0, tt - 4)

            # ---------- Phase B (attention) + AllGather + Phase C (o_proj) ----------
            with (
                tc.tile_pool(name="workB", bufs=6) as wkpool,
                tc.tile_pool(name="wC", bufs=1) as owpool,
                tc.tile_pool(name="sC", bufs=2) as cspool,
                tc.tile_pool(name="eC", bufs=3) as cepool,
                tc.tile_pool(name="psS", bufs=4, space="PSUM") as psS,
                tc.tile_pool(name="psO", bufs=2, space="PSUM") as psO,
                tc.tile_pool(name="psR", bufs=2, space="PSUM") as psR,
            ):
                # o_proj weights cached for phase C
                ow = owpool.tile([128, CT, FL], BF16, name="ow")
                nc.scalar.dma_start(ow[:], owT[:].rearrange("(o p) f -> p o f", p=128))

                WINS = (256, 512, S, S, S)  # per-slot ALiBi windows

                def phase_b(b):
                    for hl in range(HL):
                        if (b, hl) not in bt:
                            load_b(b, hl)
                        kTt, qTt, vt = bt.pop((b, hl))
                        win = WINS[hl]
                        for j in range(S // 512):  # q-tiles of 512
                            nkc = 4 * (j + 1)     # causal: k-chunks 0..4j+3
                            i0 = i_min(j, win)
                            po = psO.tile([128, 512], F32, tag="po",
                                          name=f"po{hl}_{b}_{j}")
                            pr = psR.tile([128, 512], F32, tag="pr",
                                          name=f"pr{hl}_{b}_{j}")
                            for i in range(i0, nkc):
                                ps = psS.tile([128, 512], F32, tag="ps",
                                              name=f"psB{hl}_{b}_{j}_{i}")
                                nc.tensor.matmul(
                                    ps[:],
                                    kTt[:, 128 * i:128 * (i + 1)],
                                    qTt[:, 512 * j:512 * (j + 1)],
                                    start=True, stop=(hl >= 2),
                                )
                                if hl < 2:
                                    # windowed slots: per-q shift via K=1 matmul
                                    nc.tensor.matmul(
                                        ps[:],
                                        onesM_sb[32 * hl:32 * hl + 1, :],
                                        rvs[32 * hl:32 * hl + 1,
                                            512 * j:512 * (j + 1)],
                                        start=False, stop=True,
                                    )
                                if i >= 4 * j:  # diagonal: additive causal mask
                                    tmp = wkpool.tile([128, 512], F32, tag="tmp",
                                                      name=f"tmp{hl}_{b}_{j}_{i}")
                                    nc.vector.tensor_add(
                                        tmp[:], ps[:], masks_sb[:, i - 4 * j, :])
                                    exp_in = tmp
                                else:
                                    exp_in = ps
                                pt = wkpool.tile([128, 512], BF16, tag="pt",
                                                 name=f"pt{hl}_{b}_{j}_{i}")
                                nc.scalar.activation(
                                    pt[:], exp_in[:],
                                    mybir.ActivationFunctionType.Exp,
                                    bias=bcs[:, hl, 4 * i + j:4 * i + j + 1],
                                    scale=1.0,
                                )
                                nc.tensor.matmul(
                                    po[:], vt[:, i, :], pt[:],
                                    start=(i == i0), stop=(i == nkc - 1),
                                )
                                nc.tensor.matmul(
                                    pr[:], onesM_sb[:], pt[:],
                                    start=(i == i0), stop=(i == nkc - 1),
                                )
                            recip = wkpool.tile([128, 512], F32, tag="recip",
                                                name=f"recip{hl}_{b}_{j}")
                            nc.vector.reciprocal_approx_fast(recip[:], pr[:])
                            ao = wkpool.tile([128, 512], BF16, tag="ao",
                                             name=f"ao{hl}_{b}_{j}")
                            nc.vector.tensor_mul(ao[:], po[:], recip[:])
                            nc.gpsimd.dma_start(
                                attnT_local[b][128 * hl:128 * (hl + 1),
                                               512 * j:512 * (j + 1)],
                                ao[:],
                            )

                def allgather(b):
                    nc.gpsimd.collective_compute(
                        "AllGather",
                        mybir.AluOpType.bypass,
                        ins=[attnT_local[b][:]],
                        outs=[attnT_full[b][:]],
                        replica_groups=[list(range(N_CORES))],
                    )

                def phase_c(b):
                    for tt in range(S // 256):
                        slab = cspool.tile([128, CT, 256], BF16, tag="slabC",
                                           name=f"slabC{b}_{tt}")
                        nc.sync.dma_start(
                            slab[:],
                            attnT_full[b][:, 256 * tt:256 * (tt + 1)].rearrange(
                                "(o p) t -> p o t", p=128),
                        )
                        for tc2 in range(2):
                            ev = cepool.tile([128, FL], F32, tag="evC",
                                             name=f"evC{b}_{tt}_{tc2}")
                            for seg, olo, ohi in ((0, 0, 512), (1, 512, FL)):
                                psc = psS.tile([128, 512], F32, tag="ps",
                                               name=f"psc{b}_{tt}_{tc2}_{seg}")
                                pw = ohi - olo
                                for fc in range(CT):
                                    nc.tensor.matmul(
                                        psc[:, 0:pw],
                                        slab[:, fc, 128 * tc2:128 * (tc2 + 1)],
                                        ow[:, fc, olo:ohi],
                                        start=(fc == 0), stop=(fc == CT - 1),
                                    )
                                nc.scalar.copy(ev[:, olo:ohi], psc[:, 0:pw])
                            row = S * b + 256 * tt + 128 * tc2
                            nc.gpsimd.dma_start(out[row:row + 128, :], ev[:])

                phase_b(0)
                allgather(0)
                phase_b(1)
                phase_c(0)
                allgather(1)
                phase_c(1)

    return nc


_NC = None


def _get_nc():
    global _NC
    if _NC is None:
        nc = _build_nc()
        nc.finalize()
        _NC = nc
    return _NC


def _prep_in_maps(hidden_states, w_pack, o_proj_w):
    slopes = np.asarray(_alibi_slopes(H), dtype=np.float64)
    hT = np.ascontiguousarray(hidden_states.T).astype(NPBF16)

    # Rank heads by ALiBi window (ascending) and deal them round-robin:
    # core c, slot s gets head R[8*s + c]. Must match WINS in _build_nc.
    wins = np.minimum(124.0 / slopes, float(S))
    R = np.argsort(wins, kind="stable")
    slot_wins = [256, 512, S, S, S]
    for sidx in range(HL):
        cls = wins[R[8 * sidx: 8 * (sidx + 1)]]
        assert cls.max() <= slot_wins[sidx], (sidx, cls.max())

    # shared constants
    kk = np.arange(128)
    qq = np.arange(512)
    masks = np.zeros((4, 128, 512), dtype=np.float32)
    for m in range(4):
        masks[m] = np.where((128 * m + kk)[:, None] <= qq[None, :], 0.0, -1e9
                            ).astype(np.float32)
    onesM = np.ones((128, 128), dtype=NPBF16)

    # global feature permutation induced by the head deal (for o_proj columns)
    feat_perm = np.empty(H * D, dtype=np.int64)
    for c2 in range(N_CORES):
        for sidx in range(HL):
            g0 = c2 * FL + sidx * D
            feat_perm[g0:g0 + D] = R[8 * sidx + c2] * D + np.arange(D)

    NKC = S // 128
    in_maps = []
    for c in range(N_CORES):
        heads = [int(R[8 * sidx + c]) for sidx in range(HL)]
        fsl = slice(FL * c, FL * (c + 1))
        q_rows = np.concatenate(
            [w_pack[h * D:(h + 1) * D].astype(np.float32) * SCALE for h in heads], axis=0)
        k_rows = np.concatenate(
            [w_pack[HID + h * D: HID + (h + 1) * D] for h in heads], axis=0)
        v_rows = np.concatenate(
            [w_pack[2 * HID + h * D: 2 * HID + (h + 1) * D] for h in heads], axis=0)
        wqkT = np.ascontiguousarray(
            np.concatenate([q_rows, k_rows], axis=0).T
        ).astype(NPBF16)
        wvT = np.ascontiguousarray(v_rows.T).astype(NPBF16)
        owT = np.ascontiguousarray(o_proj_w[fsl][:, feat_perm].T).astype(NPBF16)

        sl = slopes[heads]
        qpos = np.arange(S, dtype=np.float64)
        rowvec = np.ascontiguousarray(
            (-sl[:, None] * qpos[None, :])).astype(NPBF16)
        # exp bias per (k-chunk i, q-tile j): slot<2 -> slope*(128i+kk);
        # slot>=2 -> slope*(128i+kk-512j-256) (column-constant shift, cancels
        # in the softmax normalization).
        ii = np.arange(NKC, dtype=np.float64)
        jj = np.arange(4, dtype=np.float64)
        base = 128.0 * ii[None, :, None] + kk[:, None, None]      # [128, NKC, 1]
        shift = np.zeros((HL, 1, 1, 4))
        for sidx in range(HL):
            if sidx >= 2:
                shift[sidx, 0, 0, :] = 512.0 * jj + 256.0
        biascol = (sl[:, None, None, None]
                   * (base[None] - shift)).astype(np.float32)      # [HL,128,NKC,4]
        biascol = biascol.reshape(HL, 128, NKC * 4)

        in_maps.append({
            "hT": hT,
            "wqkT": wqkT,
            "wvT": wvT,
            "owT": owT,
            "rowvec": rowvec,
            "biascol": np.ascontiguousarray(biascol),
            "masks": masks,
            "onesM": onesM,
        })
    return in_maps


def _run(hidden_states, w_pack, o_proj_w, trace=False):
    global LAST_EXEC_NS
    nc = _get_nc()
    in_maps = _prep_in_maps(hidden_states, w_pack, o_proj_w)
    res = run_bass_kernel_spmd(
        nc, in_maps, core_ids=list(range(N_CORES)), trace=trace
    )
    LAST_EXEC_NS = res.exec_time_ns
    out = np.concatenate([res.results[c]["out"] for c in range(N_CORES)], axis=1)
    return np.ascontiguousarray(out.astype(np.float32))


def kernel(hidden_states, w_pack, o_proj_w, k_cache, v_cache, block_offsets,
           **_ignored):
    # The paged cache roundtrip (zero-filled caches + injective arange block
    # table, written then gathered with the same offsets) is an identity, so
    # k_cache / v_cache / block_offsets do not affect the output.
    hidden_states = np.asarray(hidden_states, dtype=np.float32)
    w_pack = np.asarray(w_pack, dtype=np.float32)
    o_proj_w = np.asarray(o_proj_w, dtype=np.float32)
    return _run(hidden_states, w_pack, o_proj_w, trace=False)


def kernel_traced(hidden_states, w_pack, o_proj_w, k_cache=None, v_cache=None,
                  block_offsets=None, **_ignored):
    hidden_states = np.asarray(hidden_states, dtype=np.float32)
    w_pack = np.asarray(w_pack, dtype=np.float32)
    o_proj_w = np.asarray(o_proj_w, dtype=np.float32)
    return _run(hidden_states, w_pack, o_proj_w, trace=True)


# revision 6
# speedup vs baseline: 1.3404x; 1.3404x over previous
"""Baichuan-13B attention block (QKV packed proj + ALiBi causal attention via
identity paged-KV roundtrip + o_proj), tensor-parallel over 8 TRN2 NeuronCores.

Sharding: heads are split 5-per-core (w_pack column shards per interleaved
q/k/v head groups, o_proj row shards); attention outputs are AllGathered
per-batch in a feature-major (D-major / transposed) layout, and each core
computes a disjoint 640-column slice of the final output, concatenated on the
host.

The paged-KV cache fill + gather in the reference is an identity mapping:
the caches start zeroed, the block table (fill=arange) is injective, and the
gather reads back exactly the freshly written K/V. So attention consumes the
projected K/V directly.

All matmuls run in bf16 (fp32 PSUM accumulation). Softmax uses the exact
max-free rewrite exp(s + slope*(k-q)). For the three full-window head slots
the per-q shift is replaced by a per-(k-chunk, q-tile) CONSTANT shift
slope*(128i + kk - 512j - 256) folded into the exp's per-partition bias: the
column-constant part cancels in the softmax normalization, so no extra matmul
is needed and the fp32/bf16 range stays bounded (|arg| <= ~63). For the two
windowed slots (large slopes) the per-q shift -slope*q is injected into the
scores PSUM by a K=1 bf16 broadcast matmul (ones^T x rowvec) as before. The
causal mask is additive (-1e9) on diagonal blocks, applied pre-exp on DVE.

ALiBi sparsity: for slope s, keys further than ~124/s behind the query
underflow to exactly 0 in fp32 exp (both here and in the reference), so those
score blocks are skipped. Since the SPMD graph is shared by all cores, heads
are ranked by their window and dealt round-robin so every core holds one head
from each of 5 window classes; the per-slot windows are hardcoded and the
host permutes w_pack head shards / o_proj columns to match.

Scheduling: QKV projection is a single merged pass over hT slabs (both weight
sets resident; slabs stream in 8-contraction-chunk pieces). Stores are issued
from the scalar (projection) and gpsimd (attention/o_proj) DGE queues so the
sync-queue loads of the next phase are not FIFO-blocked behind them.
"""

import math

import numpy as np
import ml_dtypes

import concourse.bass as bass
import concourse.mybir as mybir
import concourse.tile as tile
from concourse import bacc
from concourse.bass_utils import run_bass_kernel_spmd

# ---- problem constants (hardcoded per contract) ----
B, S = 2, 2048
HID, H, D = 5120, 40, 128
N_CORES = 8
HL = H // N_CORES            # 5 local heads
FL = HL * D                  # 640 local features
T = B * S                    # 4096 tokens
SCALE = 1.0 / math.sqrt(D)

BF16 = mybir.dt.bfloat16
F32 = mybir.dt.float32
NPBF16 = ml_dtypes.bfloat16

LAST_EXEC_NS = None


def _alibi_slopes(n):
    def pow2_slopes(m):
        start = 2.0 ** (-(2.0 ** -(math.log2(m) - 3)))
        return [start * (start ** i) for i in range(m)]
    if math.log2(n).is_integer():
        return pow2_slopes(int(n))
    m = 2 ** math.floor(math.log2(n))
    return pow2_slopes(m) + pow2_slopes(2 * m)[0::2][: n - m]


def _build_nc():
    nc = bacc.Bacc(num_devices=N_CORES)

    hT = nc.declare_dram_parameter("hT", [HID, T], BF16, isOutput=False)
    wqkT = nc.declare_dram_parameter("wqkT", [HID, 2 * FL], BF16, isOutput=False)
    wvT = nc.declare_dram_parameter("wvT", [HID, FL], BF16, isOutput=False)
    owT = nc.declare_dram_parameter("owT", [HID, FL], BF16, isOutput=False)
    rowvec = nc.declare_dram_parameter("rowvec", [HL, S], BF16, isOutput=False)
    biascol = nc.declare_dram_parameter(
        "biascol", [HL, 128, (S // 128) * 4], F32, isOutput=False)
    masks = nc.declare_dram_parameter("masks", [4, 128, 512], F32, isOutput=False)
    onesM = nc.declare_dram_parameter("onesM", [128, 128], BF16, isOutput=False)
    out = nc.declare_dram_parameter("out", [T, FL], F32, isOutput=True)

    # internal DRAM scratch
    qkT = nc.dram_tensor("qkT", [2 * FL, T], BF16)          # rows: [q feats | k feats]
    # V per head, already in the B-phase SBUF layout: [hl, b, part, outer, D]
    vtok = nc.dram_tensor("vtok", [HL, B, 128, S // 128, D], BF16)
    attnT_local = [nc.dram_tensor(f"attnT_local{b}", [FL, S], BF16) for b in range(B)]
    attnT_full = [
        nc.dram_tensor(f"attnT_full{b}", [H * D, S], BF16, addr_space="Shared")
        for b in range(B)
    ]

    CT = HID // 128  # 40 contraction chunks
    NTT = T // 512   # 8 token tiles of 512
    NKC = S // 128   # 16 k-chunks per sequence
    NCH = 5          # slab chunks per token tile
    CC = CT // NCH   # 8 contraction chunks per slab chunk

    def i_min(j, win):
        if win >= S:
            return 0
        return max(0, -(-(512 * j - win - 127) // 128))

    with tile.TileContext(nc) as tc:
        # ---------- Phase A: merged Q+K+V projection ----------
        with (
            tc.tile_pool(name="wA", bufs=1) as wpool,
            tc.tile_pool(name="sA", bufs=6) as spool,
            tc.tile_pool(name="pA", bufs=4, space="PSUM") as ppool,
            tc.tile_pool(name="pV", bufs=2, space="PSUM") as pvpool,
            tc.tile_pool(name="eA", bufs=3) as epool,
            tc.tile_pool(name="eV", bufs=2) as evpool,
        ):
            wt = wpool.tile([128, CT, 2 * FL], BF16, name="wt")
            wv = wpool.tile([128, CT, FL], BF16, name="wv")
            # chunked weight loads so the first matmuls can start early
            for qq in range(4):
                nc.scalar.dma_start(
                    wt[:, 10 * qq:10 * (qq + 1), :],
                    wqkT[1280 * qq:1280 * (qq + 1), :].rearrange(
                        "(o p) f -> p o f", p=128),
                )
            for qq in range(4):
                nc.scalar.dma_start(
                    wv[:, 10 * qq:10 * (qq + 1), :],
                    wvT[1280 * qq:1280 * (qq + 1), :].rearrange(
                        "(o p) f -> p o f", p=128),
                )
            for tt in range(NTT):
                chunks = []
                for ch in range(NCH):
                    sl = spool.tile([128, CC, 512], BF16, tag="slab",
                                    name=f"slab{tt}_{ch}")
                    nc.sync.dma_start(
                        sl[:],
                        hT[128 * CC * ch:128 * CC * (ch + 1),
                           512 * tt:512 * (tt + 1)].rearrange(
                            "(o p) t -> p o t", p=128),
                    )
                    chunks.append(sl)
                for ft in range(2 * HL):
                    ps = ppool.tile([128, 512], F32, tag="ps", name=f"psA{tt}_{ft}")
                    for ch in range(NCH):
                        for cl in range(CC):
                            ct = CC * ch + cl
                            nc.tensor.matmul(
                                ps[:],
                                wt[:, ct, 128 * ft:128 * (ft + 1)],
                                chunks[ch][:, cl, :],
                                start=(ct == 0),
                                stop=(ct == CT - 1),
                            )
                    ev = epool.tile([128, 512], BF16, tag="ev", name=f"evA{tt}_{ft}")
                    nc.scalar.copy(ev[:], ps[:])
                    nc.scalar.dma_start(
                        qkT[128 * ft:128 * (ft + 1), 512 * tt:512 * (tt + 1)],
                        ev[:],
                    )
                for tc4 in range(4):
                    psv = pvpool.tile([128, FL], F32, tag="psv", name=f"psv{tt}_{tc4}")
                    for ch in range(NCH):
                        for cl in range(CC):
                            ct = CC * ch + cl
                            nc.tensor.matmul(
                                psv[:, 0:512],
                                chunks[ch][:, cl, 128 * tc4:128 * (tc4 + 1)],
                                wv[:, ct, 0:512],
                                start=(ct == 0), stop=(ct == CT - 1),
                            )
                            nc.tensor.matmul(
                                psv[:, 512:FL],
                                chunks[ch][:, cl, 128 * tc4:128 * (tc4 + 1)],
                                wv[:, ct, 512:FL],
                                start=(ct == 0), stop=(ct == CT - 1),
                            )
                    evv = evpool.tile([128, FL], BF16, tag="evv", name=f"evv{tt}_{tc4}")
                    nc.scalar.copy(evv[:], psv[:])
                    tglob = 4 * tt + tc4
                    bb, oo = tglob // (S // 128), tglob % (S // 128)
                    for hl in range(HL):
                        nc.scalar.dma_start(
                            vtok[hl, bb, :, oo, :],
                            evv[:, 128 * hl:128 * (hl + 1)],
                        )

        # ---------- Phase B (attention) + chunked AllGather + Phase C (o_proj) ----------
        with (
            tc.tile_pool(name="constB", bufs=1) as cpool,
            tc.tile_pool(name="ioB", bufs=3) as iopool,
            tc.tile_pool(name="workB", bufs=6) as wkpool,
            tc.tile_pool(name="wC", bufs=1) as owpool,
            tc.tile_pool(name="sC", bufs=2) as cspool,
            tc.tile_pool(name="eC", bufs=3) as cepool,
            tc.tile_pool(name="psS", bufs=4, space="PSUM") as psS,
            tc.tile_pool(name="psO", bufs=2, space="PSUM") as psO,
            tc.tile_pool(name="psR", bufs=2, space="PSUM") as psR,
        ):
            masks_sb = cpool.tile([128, 4, 512], F32, name="masks_sb")
            nc.sync.dma_start(masks_sb[:], masks[:].rearrange("m p q -> p m q"))
            onesM_sb = cpool.tile([128, 128], BF16, name="onesM_sb")
            nc.sync.dma_start(onesM_sb[:], onesM[:])
            # windowed-slot row vectors at base partitions 0 and 32 (the K=1
            # matmul's stationary operand must start at partition 0/32/64)
            rvs = cpool.tile([33, S], BF16, name="rvs")
            nc.sync.dma_start(rvs[0:1, :], rowvec[0:1, :])
            nc.sync.dma_start(rvs[32:33, :], rowvec[1:2, :])
            bcs = cpool.tile([128, HL, (S // 128) * 4], F32, name="bcs")
            nc.sync.dma_start(bcs[:], biascol[:].rearrange("h p x -> p h x"))
            # o_proj weights cached for phase C
            ow = owpool.tile([128, CT, FL], BF16, name="ow")
            nc.scalar.dma_start(ow[:], owT[:].rearrange("(o p) f -> p o f", p=128))

            WINS = (256, 512, S, S, S)  # per-slot ALiBi windows (host ranks heads to match)

            def phase_b(b):
                for hl in range(HL):
                    win = WINS[hl]
                    kTt = iopool.tile([128, S], BF16, tag="kTt", name=f"kTt{hl}_{b}")
                    nc.sync.dma_start(
                        kTt[:], qkT[FL + 128 * hl: FL + 128 * (hl + 1), S * b:S * (b + 1)]
                    )
                    qTt = iopool.tile([128, S], BF16, tag="qTt", name=f"qTt{hl}_{b}")
                    nc.sync.dma_start(
                        qTt[:], qkT[128 * hl:128 * (hl + 1), S * b:S * (b + 1)]
                    )
                    vt = iopool.tile([128, NKC, D], BF16, tag="vt", name=f"vt{hl}_{b}")
                    nc.sync.dma_start(vt[:], vtok[hl, b])

                    for j in range(S // 512):  # q-tiles of 512
                        nkc = 4 * (j + 1)     # causal: k-chunks 0..4j+3
                        i0 = i_min(j, win)    # ALiBi window: earlier chunks underflow to 0
                        po = psO.tile([128, 512], F32, tag="po", name=f"po{hl}_{b}_{j}")
                        pr = psR.tile([128, 512], F32, tag="pr", name=f"pr{hl}_{b}_{j}")
                        for i in range(i0, nkc):
                            ps = psS.tile([128, 512], F32, tag="ps", name=f"psB{hl}_{b}_{j}_{i}")
                            nc.tensor.matmul(
                                ps[:],
                                kTt[:, 128 * i:128 * (i + 1)],
                                qTt[:, 512 * j:512 * (j + 1)],
                                start=True, stop=(hl >= 2),
                            )
                            if hl < 2:
                                # windowed slots: per-q shift via K=1 broadcast matmul
                                nc.tensor.matmul(
                                    ps[:],
                                    onesM_sb[32 * hl:32 * hl + 1, :],
                                    rvs[32 * hl:32 * hl + 1,
                                        512 * j:512 * (j + 1)],
                                    start=False, stop=True,
                                )
                            if i >= 4 * j:  # diagonal block: additive causal mask (-1e9)
                                tmp = wkpool.tile([128, 512], F32, tag="tmp",
                                                  name=f"tmp{hl}_{b}_{j}_{i}")
                                nc.vector.tensor_add(tmp[:], ps[:], masks_sb[:, i - 4 * j, :])
                                exp_in = tmp
                            else:
                                exp_in = ps
                            pt = wkpool.tile([128, 512], BF16, tag="pt", name=f"pt{hl}_{b}_{j}_{i}")
                            nc.scalar.activation(
                                pt[:], exp_in[:], mybir.ActivationFunctionType.Exp,
                                bias=bcs[:, hl, 4 * i + j:4 * i + j + 1],
                                scale=1.0,
                            )
                            nc.tensor.matmul(
                                po[:], vt[:, i, :], pt[:],
                                start=(i == i0), stop=(i == nkc - 1),
                            )
                            nc.tensor.matmul(
                                pr[:], onesM_sb[:], pt[:],
                                start=(i == i0), stop=(i == nkc - 1),
                            )
                        recip = wkpool.tile([128, 512], F32, tag="recip", name=f"recip{hl}_{b}_{j}")
                        nc.vector.reciprocal_approx_fast(recip[:], pr[:])
                        ao = wkpool.tile([128, 512], BF16, tag="ao", name=f"ao{hl}_{b}_{j}")
                        nc.vector.tensor_mul(ao[:], po[:], recip[:])
                        nc.gpsimd.dma_start(
                            attnT_local[b][128 * hl:128 * (hl + 1), 512 * j:512 * (j + 1)],
                            ao[:],
                        )

            def allgather(b):
                nc.gpsimd.collective_compute(
                    "AllGather",
                    mybir.AluOpType.bypass,
                    ins=[attnT_local[b][:]],
                    outs=[attnT_full[b][:]],
                    replica_groups=[list(range(N_CORES))],
                )

            def phase_c(b):
                for tt in range(S // 256):
                    slab = cspool.tile([128, CT, 256], BF16, tag="slabC", name=f"slabC{b}_{tt}")
                    nc.sync.dma_start(
                        slab[:],
                        attnT_full[b][:, 256 * tt:256 * (tt + 1)].rearrange(
                            "(o p) t -> p o t", p=128),
                    )
                    for tc2 in range(2):
                        ev = cepool.tile([128, FL], F32, tag="evC", name=f"evC{b}_{tt}_{tc2}")
                        for seg, olo, ohi in ((0, 0, 512), (1, 512, FL)):
                            psc = psS.tile([128, 512], F32, tag="ps",
                                           name=f"psc{b}_{tt}_{tc2}_{seg}")
                            pw = ohi - olo
                            for fc in range(CT):
                                nc.tensor.matmul(
                                    psc[:, 0:pw],
                                    slab[:, fc, 128 * tc2:128 * (tc2 + 1)],
                                    ow[:, fc, olo:ohi],
                                    start=(fc == 0), stop=(fc == CT - 1),
                                )
                            nc.scalar.copy(ev[:, olo:ohi], psc[:, 0:pw])
                        row = S * b + 256 * tt + 128 * tc2
                        nc.gpsimd.dma_start(out[row:row + 128, :], ev[:])

            phase_b(0)
            allgather(0)
            phase_b(1)
            phase_c(0)
            allgather(1)
            phase_c(1)

    return nc


_NC = None


def _get_nc():
    global _NC
    if _NC is None:
        nc = _build_nc()
        nc.finalize()
        _NC = nc
    return _NC


def _prep_in_maps(hidden_states, w_pack, o_proj_w):
    slopes = np.asarray(_alibi_slopes(H), dtype=np.float64)
    hT = np.ascontiguousarray(hidden_states.T).astype(NPBF16)

    # Rank heads by ALiBi window (ascending) and deal them round-robin:
    # core c, slot s gets head R[8*s + c]. Must match WINS in _build_nc:
    # slot windows bound every head in that rank octile.
    wins = np.minimum(124.0 / slopes, float(S))
    R = np.argsort(wins, kind="stable")
    slot_wins = [256, 512, S, S, S]
    for sidx in range(HL):
        cls = wins[R[8 * sidx: 8 * (sidx + 1)]]
        assert cls.max() <= slot_wins[sidx], (sidx, cls.max())

    # shared constants
    kk = np.arange(128)
    qq = np.arange(512)
    masks = np.zeros((4, 128, 512), dtype=np.float32)
    for m in range(4):
        masks[m] = np.where((128 * m + kk)[:, None] <= qq[None, :], 0.0, -1e9
                            ).astype(np.float32)
    onesM = np.ones((128, 128), dtype=NPBF16)

    # global feature permutation induced by the head deal (for o_proj columns)
    feat_perm = np.empty(H * D, dtype=np.int64)
    for c2 in range(N_CORES):
        for sidx in range(HL):
            g0 = c2 * FL + sidx * D
            feat_perm[g0:g0 + D] = R[8 * sidx + c2] * D + np.arange(D)

    NKC = S // 128
    in_maps = []
    for c in range(N_CORES):
        heads = [int(R[8 * sidx + c]) for sidx in range(HL)]
        fsl = slice(FL * c, FL * (c + 1))
        q_rows = np.concatenate(
            [w_pack[h * D:(h + 1) * D].astype(np.float32) * SCALE for h in heads], axis=0)
        k_rows = np.concatenate(
            [w_pack[HID + h * D: HID + (h + 1) * D] for h in heads], axis=0)
        v_rows = np.concatenate(
            [w_pack[2 * HID + h * D: 2 * HID + (h + 1) * D] for h in heads], axis=0)
        wqkT = np.ascontiguousarray(
            np.concatenate([q_rows, k_rows], axis=0).T
        ).astype(NPBF16)
        wvT = np.ascontiguousarray(v_rows.T).astype(NPBF16)
        owT = np.ascontiguousarray(o_proj_w[fsl][:, feat_perm].T).astype(NPBF16)

        sl = slopes[heads]
        qpos = np.arange(S, dtype=np.float64)
        rowvec = np.ascontiguousarray(
            (-sl[:, None] * qpos[None, :])).astype(NPBF16)
        # exp bias per (k-chunk i, q-tile j): slot<2 -> slope*(128i+kk);
        # slot>=2 -> slope*(128i+kk-512j-256) (column-constant shift, cancels
        # in the softmax normalization).
        ii = np.arange(NKC, dtype=np.float64)
        jj = np.arange(4, dtype=np.float64)
        base = 128.0 * ii[None, :, None] + kk[:, None, None]      # [128, NKC, 1]
        shift = np.zeros((HL, 1, 1, 4))
        for sidx in range(HL):
            if sidx >= 2:
                shift[sidx, 0, 0, :] = 512.0 * jj + 256.0
        biascol = (sl[:, None, None, None]
                   * (base[None] - shift)).astype(np.float32)      # [HL,128,NKC,4]
        biascol = biascol.reshape(HL, 128, NKC * 4)

        in_maps.append({
            "hT": hT,
            "wqkT": wqkT,
            "wvT": wvT,
            "owT": owT,
            "rowvec": rowvec,
            "biascol": np.ascontiguousarray(biascol),
            "masks": masks,
            "onesM": onesM,
        })
    return in_maps


def _run(hidden_states, w_pack, o_proj_w, trace=False):
    global LAST_EXEC_NS
    nc = _get_nc()
    in_maps = _prep_in_maps(hidden_states, w_pack, o_proj_w)
    res = run_bass_kernel_spmd(
        nc, in_maps, core_ids=list(range(N_CORES)), trace=trace
    )
    LAST_EXEC_NS = res.exec_time_ns
    out = np.concatenate([res.results[c]["out"] for c in range(N_CORES)], axis=1)
    return np.ascontiguousarray(out.astype(np.float32))


def kernel(hidden_states, w_pack, o_proj_w, k_cache, v_cache, block_offsets,
           **_ignored):
    # The paged cache roundtrip (zero-filled caches + injective arange block
    # table, written then gathered with the same offsets) is an identity, so
    # k_cache / v_cache / block_offsets do not affect the output.
    hidden_states = np.asarray(hidden_states, dtype=np.float32)
    w_pack = np.asarray(w_pack, dtype=np.float32)
    o_proj_w = np.asarray(o_proj_w, dtype=np.float32)
    return _run(hidden_states, w_pack, o_proj_w, trace=False)


def kernel_traced(hidden_states, w_pack, o_proj_w, k_cache=None, v_cache=None,
                  block_offsets=None, **_ignored):
    hidden_states = np.asarray(hidden_states, dtype=np.float32)
    w_pack = np.asarray(w_pack, dtype=np.float32)
    o_proj_w = np.asarray(o_proj_w, dtype=np.float32)
    return _run(hidden_states, w_pack, o_proj_w, trace=True)
